# revision 12
# baseline (speedup 1.0000x reference)
"""Single-directional Chamfer distance on 8 Trainium2 NeuronCores.

Problem: v, v_pred: [4, 8192, 3] f32.
  out = mean_b mean_i min_j ||v_pred[b,i] - v[b,j]||^2   (scalar f32)

Sharding: 8 cores = 4 batches x 2 halves of the v_pred point axis.
Per core: x = v_pred[b, h*4096:(h+1)*4096] (4096 pts), y = v[b] (8192 pts).

The PE computes squared distances directly as a matmul over an augmented
contraction dim: conceptually
  lhsT rows = [-2*x, |x|^2, 1]  (stationary, 128 x-points per tile)
  rhs  rows = [y, 1, |y|^2]     (moving, 512-col chunks)
  -> psum[i, j] = |x_i - y_j|^2
realized as an error-compensated K=13 bf16 split (fp32 matmuls stream at
1/4 the rate of bf16 on the PE; see the comment in _build_program), so the
pairwise distances are fp32-accurate to ~2e-5 absolute.  All rows are
built on device from the raw coords; the [128, grid] compute layouts
bounce through a DRAM scratch so one strided DMA can deliver the [K, n]
row layout (SBUF APs cannot iterate the partition dim innermost; DRAM APs
can).

The min over j per x-tile (4 PSUM groups of [128, 2048]): group 0 is
min-reduced in fp32 straight from PSUM by the DVE; the otherwise-idle
ScalarE casts groups 1-3 to bf16 in SBUF (values are true squared
distances, so bf16 rounding is benign) and the DVE folds them with bf16
tensor_tensor mins at 2 elem/cycle.  Per-core output: [128, 32] min
distances; the host takes the float64 mean of all 8 cores' outputs.

Built on bacc.Bacc + nc.compile(): walrus allows at most ~1 embedded sync
wait per instruction, and bacc's generate_event_semaphores() legalizes
multi-producer waits.  tensor_tensor_reduce is avoided entirely — it
compiles and simulates but faults at runtime on this stack.
"""

import numpy as np

import concourse.bacc as bacc
import concourse.bass as bass
import concourse.mybir as mybir
import concourse.tile as tile
from concourse.bass_utils import run_bass_kernel_spmd

F32 = mybir.dt.float32

B = 4            # batches
NPTS = 8192      # v_pred points per batch
MPTS = 8192      # v points per batch
NCORES = 8
XS = NPTS // 2   # x points per core
XTILES = XS // 128          # 32 x-tiles of 128
YC = 512                    # matmul moving chunk (fp32 max)
GCOLS = 2048                # psum group columns (4 banks)
NGROUP = MPTS // GCOLS      # 4 groups per x-tile
XGT = XS // 128             # 32: x-grid minor dim
YGT = MPTS // 128           # 64: y-grid minor dim

_built = None


def _build_program():
    nc = bacc.Bacc(None, target_bir_lowering=False)
    xl_d = nc.declare_dram_parameter("xl", [128, XGT * 3], F32, isOutput=False)
    yl_d = nc.declare_dram_parameter("yl", [128, YGT * 3], F32, isOutput=False)
    out_d = nc.declare_dram_parameter("out", [128, XTILES], F32, isOutput=True)

    # DRAM bounce scratch for the row-layout remaps
    BF = mybir.dt.bfloat16
    KK = 13   # split-bf16 contraction rows (see below)
    xs_d = nc.dram_tensor("xstage", [128, XGT * KK], BF)
    ys_d = nc.dram_tensor("ystage", [128, YGT * KK], BF)

    with tile.TileContext(nc) as tc:
        with (
            tc.tile_pool(name="const", bufs=1) as cp,
            tc.tile_pool(name="gm", bufs=3) as gp,
            tc.tile_pool(name="ps", bufs=2, space="PSUM") as pp,
        ):
            xl_sb = cp.tile([128, XGT * 3], F32)
            yl_sb = cp.tile([128, YGT * 3], F32)
            xt_sb = cp.tile([KK, XS], BF)      # lhsT rows
            rhs = cp.tile([KK, MPTS], BF)      # moving rows
            nc.sync.dma_start(out=xl_sb[:], in_=xl_d[:])
            nc.sync.dma_start(out=yl_sb[:], in_=yl_d[:])

            # fp32 matmuls cost ~853ns/MM on the PE (no FWL, half-rate
            # streaming) vs ~213ns for bf16.  So the K=5 fp32 contraction is
            # replaced by an error-compensated K=13 bf16 split:
            #   x = xh + xl, y = yh + yl (exact bf16 hi/lo pairs; scaling by
            #   -2 is exact), keeping the hh + hl + lh product terms, and
            #   x^2, y^2 as exact bf16 pairs against ones:
            #     k=3d+0: -2*xh_d * yh_d      k=9:  x2h * 1
            #     k=3d+1: -2*xh_d * yl_d      k=10: x2l * 1
            #     k=3d+2: -2*xl_d * yh_d      k=11: 1 * y2h
            #                                 k=12: 1 * y2l
            #   dropped: xl*yl terms ~2^-18*|x||y| (~2e-5 absolute on d2).

            def build_split_grid(src_sb, gt, sq_rows_first):
                """src_sb: [128, gt*3] f32 coords.  Returns [128, gt*KK] bf16
                staging grid.  sq_rows_first=True -> rows 9,10 = (sq_h, sq_l)
                and 11,12 = ones (the x side); False -> rows 9,10 = ones and
                11,12 = (sq_h, sq_l) (the y side).  For the x side the coord
                rows carry -2*(hi/lo); for the y side the raw hi/lo."""
                pre = "x" if sq_rows_first else "y"
                hi = cp.tile([128, gt * 3], BF, name=f"{pre}hi")
                nc.vector.tensor_copy(out=hi[:], in_=src_sb[:])
                res = cp.tile([128, gt * 3], F32, name=f"{pre}res")
                nc.vector.tensor_sub(out=res[:], in0=src_sb[:], in1=hi[:])
                lo = cp.tile([128, gt * 3], BF, name=f"{pre}lo")
                nc.vector.tensor_copy(out=lo[:], in_=res[:])
                if sq_rows_first:
                    # fold the exact -2 into both halves
                    m2h = cp.tile([128, gt * 3], BF, name=f"{pre}m2h")
                    nc.vector.tensor_scalar_mul(out=m2h[:], in0=hi[:], scalar1=-2.0)
                    m2l = cp.tile([128, gt * 3], BF, name=f"{pre}m2l")
                    nc.vector.tensor_scalar_mul(out=m2l[:], in0=lo[:], scalar1=-2.0)
                    hi, lo = m2h, m2l
                # squared norms from the full fp32 coords
                sq3 = cp.tile([128, gt * 3], F32, name=f"{pre}sq3")
                nc.vector.tensor_mul(out=sq3[:], in0=src_sb[:], in1=src_sb[:])
                sq = cp.tile([128, gt], F32, name=f"{pre}sq")
                nc.vector.tensor_reduce(
                    out=sq[:], in_=sq3.rearrange("p (t d) -> p t d", d=3),
                    axis=mybir.AxisListType.X, op=mybir.AluOpType.add,
                )
                sqh = cp.tile([128, gt], BF, name=f"{pre}sqh")
                nc.vector.tensor_copy(out=sqh[:], in_=sq[:])
                sqr = cp.tile([128, gt], F32, name=f"{pre}sqr")
                nc.vector.tensor_sub(out=sqr[:], in0=sq[:], in1=sqh[:])
                sql = cp.tile([128, gt], BF, name=f"{pre}sql")
                nc.vector.tensor_copy(out=sql[:], in_=sqr[:])

                grid = cp.tile([128, gt * KK], BF, name=f"{pre}grid")
                gv = grid.rearrange("p (t k) -> p t k", k=KK)
                hv = hi.rearrange("p (t d) -> p t d", d=3)
                lv = lo.rearrange("p (t d) -> p t d", d=3)
                for d in range(3):
                    if sq_rows_first:   # x side: (-2xh, -2xh, -2xl)
                        nc.vector.tensor_copy(out=gv[:, :, 3 * d], in_=hv[:, :, d])
                        nc.vector.tensor_copy(out=gv[:, :, 3 * d + 1], in_=hv[:, :, d])
                        nc.vector.tensor_copy(out=gv[:, :, 3 * d + 2], in_=lv[:, :, d])
                    else:               # y side: (yh, yl, yh)
                        nc.vector.tensor_copy(out=gv[:, :, 3 * d], in_=hv[:, :, d])
                        nc.vector.tensor_copy(out=gv[:, :, 3 * d + 1], in_=lv[:, :, d])
                        nc.vector.tensor_copy(out=gv[:, :, 3 * d + 2], in_=hv[:, :, d])
                if sq_rows_first:
                    nc.vector.tensor_copy(out=gv[:, :, 9], in_=sqh[:])
                    nc.vector.tensor_copy(out=gv[:, :, 10], in_=sql[:])
                    one_a, one_b = 11, 12
                else:
                    nc.vector.tensor_copy(out=gv[:, :, 11], in_=sqh[:])
                    nc.vector.tensor_copy(out=gv[:, :, 12], in_=sql[:])
                    one_a, one_b = 9, 10
                for k in (one_a, one_b):
                    nc.vector.tensor_scalar(
                        out=gv[:, :, k], in0=sqh[:], scalar1=0.0, scalar2=1.0,
                        op0=mybir.AluOpType.mult, op1=mybir.AluOpType.add,
                    )
                return grid

            xg = build_split_grid(xl_sb, XGT, True)
            nc.sync.dma_start(out=xs_d[:], in_=xg[:])
            nc.sync.dma_start(
                out=xt_sb[:], in_=xs_d.rearrange("p (t k) -> k (p t)", k=KK)
            )
            yg = build_split_grid(yl_sb, YGT, False)
            nc.sync.dma_start(out=ys_d[:], in_=yg[:])
            nc.sync.dma_start(
                out=rhs[:], in_=ys_d.rearrange("p (t k) -> k (p t)", k=KK)
            )

            # Drain: group 0 is min-reduced in fp32 straight from PSUM by
            # the DVE (1 elem/cycle).  Groups 1-3 are cast to bf16 in SBUF
            # by the otherwise-idle ScalarE (the PSUM values are true
            # squared distances, so bf16 rounding costs only ~0.4% of the
            # tiny d2 values, ~1e-5 absolute on the output) and folded by
            # bf16 tensor_tensor mins, which run at 2 elem/cycle.
            BF = mybir.dt.bfloat16
            dmin = cp.tile([128, XTILES], F32)
            for t in range(XTILES):
                lhsT = xt_sb[:, t * 128:(t + 1) * 128]
                gm = gp.tile([128, 2], F32, tag="gm", name="gm")
                cbs = []
                for g in range(NGROUP):
                    ps = pp.tile([128, GCOLS], F32, tag="ps", name="ps")
                    for c in range(GCOLS // YC):
                        j0 = g * GCOLS + c * YC
                        nc.tensor.matmul(
                            out=ps[:, c * YC:(c + 1) * YC],
                            lhsT=lhsT, rhs=rhs[:, j0:j0 + YC],
                        )
                    if g == 0:
                        nc.vector.tensor_reduce(
                            out=gm[:, 0:1], in_=ps[:],
                            axis=mybir.AxisListType.X, op=mybir.AluOpType.min,
                        )
                    else:
                        cb = gp.tile([128, GCOLS], BF, tag="cb", name="cb",
                                     bufs=6)
                        nc.scalar.copy(out=cb[:], in_=ps[:])
                        cbs.append(cb)
                b12 = gp.tile([128, GCOLS], BF, tag="bt", name="b12")
                nc.vector.tensor_tensor(out=b12[:], in0=cbs[0][:], in1=cbs[1][:],
                                        op=mybir.AluOpType.min)
                b123 = gp.tile([128, GCOLS], BF, tag="bt", name="b123")
                nc.vector.tensor_tensor(out=b123[:], in0=b12[:], in1=cbs[2][:],
                                        op=mybir.AluOpType.min)
                h1 = gp.tile([128, GCOLS // 2], BF, tag="h1", name="h1")
                nc.vector.tensor_tensor(out=h1[:], in0=b123[:, :GCOLS // 2],
                                        in1=b123[:, GCOLS // 2:],
                                        op=mybir.AluOpType.min)
                h2 = gp.tile([128, GCOLS // 4], BF, tag="h2", name="h2")
                nc.vector.tensor_tensor(out=h2[:], in0=h1[:, :GCOLS // 4],
                                        in1=h1[:, GCOLS // 4:],
                                        op=mybir.AluOpType.min)
                nc.vector.tensor_reduce(
                    out=gm[:, 1:2], in_=h2[:],
                    axis=mybir.AxisListType.X, op=mybir.AluOpType.min,
                )
                nc.vector.tensor_reduce(
                    out=dmin[:, t:t + 1], in_=gm[:],
                    axis=mybir.AxisListType.X, op=mybir.AluOpType.min,
                )

            nc.sync.dma_start(out=out_d[:], in_=dmin[:])

    # bacc compile: splits multi-sem waits into EventSemaphore insts
    # (walrus allows at most 1 embedded wait per instruction), fuses nops,
    # allocates registers.
    nc.compile()
    return nc


def _shard_inputs(v, v_pred):
    v = np.asarray(v, dtype=np.float32)
    v_pred = np.asarray(v_pred, dtype=np.float32)
    in_maps = []
    for c in range(NCORES):
        b, h = divmod(c, 2)
        xc = v_pred[b, h * XS:(h + 1) * XS]   # [4096, 3]
        y = v[b]                              # [8192, 3]
        in_maps.append({
            "xl": np.ascontiguousarray(xc.reshape(128, XGT * 3)),
            "yl": np.ascontiguousarray(y.reshape(128, YGT * 3)),
        })
    return in_maps


def _get_program():
    global _built
    if _built is None:
        _built = _build_program()
    return _built


def run_spmd(v, v_pred, **kwargs):
    """Run the SPMD program; returns BassKernelResults."""
    nc = _get_program()
    in_maps = _shard_inputs(v, v_pred)
    res = run_bass_kernel_spmd(nc, in_maps, list(range(NCORES)), **kwargs)
    return res


def kernel(v, v_pred):
    res = run_spmd(v, v_pred)
    total = 0.0
    for c in range(NCORES):
        total += np.asarray(res.results[c]["out"], dtype=np.float64).sum()
    mean = total / (B * NPTS)
    return np.array(mean, dtype=np.float32)


# revision 15
# speedup vs baseline: 1.0003x; 1.0003x over previous
"""Single-directional Chamfer distance on 8 Trainium2 NeuronCores.

Problem: v, v_pred: [4, 8192, 3] f32.
  out = mean_b mean_i min_j ||v_pred[b,i] - v[b,j]||^2   (scalar f32)

Sharding: 8 cores = 4 batches x 2 halves of the v_pred point axis.
Per core: x = v_pred[b, h*4096:(h+1)*4096] (4096 pts), y = v[b] (8192 pts).

The PE computes squared distances directly as a matmul over an augmented
contraction dim: conceptually
  lhsT rows = [-2*x, |x|^2, 1]  (stationary, 128 x-points per tile)
  rhs  rows = [y, 1, |y|^2]     (moving, 512-col chunks)
  -> psum[i, j] = |x_i - y_j|^2
realized as an error-compensated K=13 bf16 split (fp32 matmuls stream at
1/4 the rate of bf16 on the PE; see the comment in _build_program), so the
pairwise distances are fp32-accurate to ~2e-5 absolute.  All rows are
built on device from the raw coords; the [128, grid] compute layouts
bounce through a DRAM scratch so one strided DMA can deliver the [K, n]
row layout (SBUF APs cannot iterate the partition dim innermost; DRAM APs
can).

The min over j per x-tile (4 PSUM groups of [128, 2048]): group 0 is
min-reduced in fp32 straight from PSUM by the DVE; the otherwise-idle
ScalarE casts groups 1-3 to bf16 in SBUF (values are true squared
distances, so bf16 rounding is benign) and the DVE folds them with bf16
tensor_tensor mins at 2 elem/cycle.  Per-core output: [128, 32] min
distances; the host takes the float64 mean of all 8 cores' outputs.

Built on bacc.Bacc + nc.compile(): walrus allows at most ~1 embedded sync
wait per instruction, and bacc's generate_event_semaphores() legalizes
multi-producer waits.  tensor_tensor_reduce is avoided entirely — it
compiles and simulates but faults at runtime on this stack.
"""

import os

import numpy as np

import concourse.bacc as bacc
import concourse.bass as bass
import concourse.mybir as mybir
import concourse.tile as tile
from concourse.bass_utils import run_bass_kernel_spmd

F32 = mybir.dt.float32

B = 4            # batches
NPTS = 8192      # v_pred points per batch
MPTS = 8192      # v points per batch
NCORES = 8
XS = NPTS // 2   # x points per core
XTILES = int(os.environ.get("CHAMFER_XTILES", str(XS // 128)))
YC = int(os.environ.get("CHAMFER_YC", "512"))   # matmul moving chunk
GCOLS = 2048                # psum group columns (4 banks)
NGROUP = MPTS // GCOLS      # 4 groups per x-tile
XGT = XS // 128             # 32: x-grid minor dim
YGT = MPTS // 128           # 64: y-grid minor dim

_built = None


def _build_program():
    nc = bacc.Bacc(None, target_bir_lowering=False)
    xl_d = nc.declare_dram_parameter("xl", [128, XGT * 3], F32, isOutput=False)
    yl_d = nc.declare_dram_parameter("yl", [128, YGT * 3], F32, isOutput=False)
    out_d = nc.declare_dram_parameter("out", [128, XTILES], F32, isOutput=True)

    # DRAM bounce scratch for the row-layout remaps
    BF = mybir.dt.bfloat16
    KK = 13   # split-bf16 contraction rows (see below)
    xs_d = nc.dram_tensor("xstage", [128, XGT * KK], BF)
    ys_d = nc.dram_tensor("ystage", [128, YGT * KK], BF)

    with tile.TileContext(nc) as tc:
        with (
            tc.tile_pool(name="const", bufs=1) as cp,
            tc.tile_pool(name="gm", bufs=4) as gp,
            tc.tile_pool(name="ps", bufs=2, space="PSUM") as pp,
        ):
            xl_sb = cp.tile([128, XGT * 3], F32)
            yl_sb = cp.tile([128, YGT * 3], F32)
            xt_sb = cp.tile([KK, XS], BF)      # lhsT rows
            rhs = cp.tile([KK, MPTS], BF)      # moving rows
            nc.sync.dma_start(out=xl_sb[:], in_=xl_d[:])
            nc.sync.dma_start(out=yl_sb[:], in_=yl_d[:])

            # fp32 matmuls cost ~853ns/MM on the PE (no FWL, half-rate
            # streaming) vs ~213ns for bf16.  So the K=5 fp32 contraction is
            # replaced by an error-compensated K=13 bf16 split:
            #   x = xh + xl, y = yh + yl (exact bf16 hi/lo pairs; scaling by
            #   -2 is exact), keeping the hh + hl + lh product terms, and
            #   x^2, y^2 as exact bf16 pairs against ones:
            #     k=3d+0: -2*xh_d * yh_d      k=9:  x2h * 1
            #     k=3d+1: -2*xh_d * yl_d      k=10: x2l * 1
            #     k=3d+2: -2*xl_d * yh_d      k=11: 1 * y2h
            #                                 k=12: 1 * y2l
            #   dropped: xl*yl terms ~2^-18*|x||y| (~2e-5 absolute on d2).

            def build_split_grid(src_sb, gt, sq_rows_first):
                """src_sb: [128, gt*3] f32 coords.  Returns [128, gt*KK] bf16
                staging grid.  sq_rows_first=True -> rows 9,10 = (sq_h, sq_l)
                and 11,12 = ones (the x side); False -> rows 9,10 = ones and
                11,12 = (sq_h, sq_l) (the y side).  For the x side the coord
                rows carry -2*(hi/lo); for the y side the raw hi/lo."""
                pre = "x" if sq_rows_first else "y"
                hi = cp.tile([128, gt * 3], BF, name=f"{pre}hi")
                nc.vector.tensor_copy(out=hi[:], in_=src_sb[:])
                res = cp.tile([128, gt * 3], F32, name=f"{pre}res")
                nc.vector.tensor_sub(out=res[:], in0=src_sb[:], in1=hi[:])
                lo = cp.tile([128, gt * 3], BF, name=f"{pre}lo")
                nc.vector.tensor_copy(out=lo[:], in_=res[:])
                if sq_rows_first:
                    # fold the exact -2 into both halves
                    m2h = cp.tile([128, gt * 3], BF, name=f"{pre}m2h")
                    nc.vector.tensor_scalar_mul(out=m2h[:], in0=hi[:], scalar1=-2.0)
                    m2l = cp.tile([128, gt * 3], BF, name=f"{pre}m2l")
                    nc.vector.tensor_scalar_mul(out=m2l[:], in0=lo[:], scalar1=-2.0)
                    hi, lo = m2h, m2l
                # squared norms from the full fp32 coords
                sq3 = cp.tile([128, gt * 3], F32, name=f"{pre}sq3")
                nc.vector.tensor_mul(out=sq3[:], in0=src_sb[:], in1=src_sb[:])
                sq = cp.tile([128, gt], F32, name=f"{pre}sq")
                nc.vector.tensor_reduce(
                    out=sq[:], in_=sq3.rearrange("p (t d) -> p t d", d=3),
                    axis=mybir.AxisListType.X, op=mybir.AluOpType.add,
                )
                sqh = cp.tile([128, gt], BF, name=f"{pre}sqh")
                nc.vector.tensor_copy(out=sqh[:], in_=sq[:])
                sqr = cp.tile([128, gt], F32, name=f"{pre}sqr")
                nc.vector.tensor_sub(out=sqr[:], in0=sq[:], in1=sqh[:])
                sql = cp.tile([128, gt], BF, name=f"{pre}sql")
                nc.vector.tensor_copy(out=sql[:], in_=sqr[:])

                grid = cp.tile([128, gt * KK], BF, name=f"{pre}grid")
                gv = grid.rearrange("p (t k) -> p t k", k=KK)
                hv = hi.rearrange("p (t d) -> p t d", d=3)
                lv = lo.rearrange("p (t d) -> p t d", d=3)
                for d in range(3):
                    if sq_rows_first:   # x side: (-2xh, -2xh, -2xl)
                        nc.vector.tensor_copy(out=gv[:, :, 3 * d], in_=hv[:, :, d])
                        nc.vector.tensor_copy(out=gv[:, :, 3 * d + 1], in_=hv[:, :, d])
                        nc.vector.tensor_copy(out=gv[:, :, 3 * d + 2], in_=lv[:, :, d])
                    else:               # y side: (yh, yl, yh)
                        nc.vector.tensor_copy(out=gv[:, :, 3 * d], in_=hv[:, :, d])
                        nc.vector.tensor_copy(out=gv[:, :, 3 * d + 1], in_=lv[:, :, d])
                        nc.vector.tensor_copy(out=gv[:, :, 3 * d + 2], in_=hv[:, :, d])
                if sq_rows_first:
                    nc.vector.tensor_copy(out=gv[:, :, 9], in_=sqh[:])
                    nc.vector.tensor_copy(out=gv[:, :, 10], in_=sql[:])
                    one_a, one_b = 11, 12
                else:
                    nc.vector.tensor_copy(out=gv[:, :, 11], in_=sqh[:])
                    nc.vector.tensor_copy(out=gv[:, :, 12], in_=sql[:])
                    one_a, one_b = 9, 10
                for k in (one_a, one_b):
                    nc.vector.tensor_scalar(
                        out=gv[:, :, k], in0=sqh[:], scalar1=0.0, scalar2=1.0,
                        op0=mybir.AluOpType.mult, op1=mybir.AluOpType.add,
                    )
                return grid

            xg = build_split_grid(xl_sb, XGT, True)
            nc.sync.dma_start(out=xs_d[:], in_=xg[:])
            nc.sync.dma_start(
                out=xt_sb[:], in_=xs_d.rearrange("p (t k) -> k (p t)", k=KK)
            )
            yg = build_split_grid(yl_sb, YGT, False)
            nc.sync.dma_start(out=ys_d[:], in_=yg[:])
            nc.sync.dma_start(
                out=rhs[:], in_=ys_d.rearrange("p (t k) -> k (p t)", k=KK)
            )

            # Drain: group 0 is min-reduced in fp32 straight from PSUM by
            # the DVE (1 elem/cycle).  Groups 1-3 are cast to bf16 in SBUF
            # by the otherwise-idle ScalarE (the PSUM values are true
            # squared distances, so bf16 rounding costs only ~0.4% of the
            # tiny d2 values, ~1e-5 absolute on the output) and folded by
            # bf16 tensor_tensor mins, which run at 2 elem/cycle.
            BF = mybir.dt.bfloat16
            dmin = cp.tile([128, XTILES], F32)
            for t in range(XTILES):
                lhsT = xt_sb[:, t * 128:(t + 1) * 128]
                gm = gp.tile([128, 2], F32, tag="gm", name="gm")
                cbs = []
                for g in (1, 2, 3, 0):
                    ps = pp.tile([128, GCOLS], F32, tag="ps", name="ps")
                    for c in range(GCOLS // YC):
                        j0 = g * GCOLS + c * YC
                        nc.tensor.matmul(
                            out=ps[:, c * YC:(c + 1) * YC],
                            lhsT=lhsT, rhs=rhs[:, j0:j0 + YC],
                        )
                    if g == 0:
                        nc.vector.tensor_reduce(
                            out=gm[:, 0:1], in_=ps[:],
                            axis=mybir.AxisListType.X, op=mybir.AluOpType.min,
                        )
                    else:
                        cb = gp.tile([128, GCOLS], BF, tag="cb", name="cb",
                                     bufs=10)
                        nc.scalar.copy(out=cb[:], in_=ps[:])
                        cbs.append(cb)
                b12 = gp.tile([128, GCOLS], BF, tag="bt", name="b12")
                nc.vector.tensor_tensor(out=b12[:], in0=cbs[0][:], in1=cbs[1][:],
                                        op=mybir.AluOpType.min)
                b123 = gp.tile([128, GCOLS], BF, tag="bt", name="b123")
                nc.vector.tensor_tensor(out=b123[:], in0=b12[:], in1=cbs[2][:],
                                        op=mybir.AluOpType.min)
                h1 = gp.tile([128, GCOLS // 2], BF, tag="h1", name="h1")
                nc.vector.tensor_tensor(out=h1[:], in0=b123[:, :GCOLS // 2],
                                        in1=b123[:, GCOLS // 2:],
                                        op=mybir.AluOpType.min)
                h2 = gp.tile([128, GCOLS // 4], BF, tag="h2", name="h2")
                nc.vector.tensor_tensor(out=h2[:], in0=h1[:, :GCOLS // 4],
                                        in1=h1[:, GCOLS // 4:],
                                        op=mybir.AluOpType.min)
                h3 = gp.tile([128, GCOLS // 8], BF, tag="h3", name="h3")
                nc.vector.tensor_tensor(out=h3[:], in0=h2[:, :GCOLS // 8],
                                        in1=h2[:, GCOLS // 8:],
                                        op=mybir.AluOpType.min)
                nc.vector.tensor_reduce(
                    out=gm[:, 1:2], in_=h3[:],
                    axis=mybir.AxisListType.X, op=mybir.AluOpType.min,
                )
                nc.vector.tensor_reduce(
                    out=dmin[:, t:t + 1], in_=gm[:],
                    axis=mybir.AxisListType.X, op=mybir.AluOpType.min,
                )

            nc.sync.dma_start(out=out_d[:], in_=dmin[:])

    # bacc compile: splits multi-sem waits into EventSemaphore insts
    # (walrus allows at most 1 embedded wait per instruction), fuses nops,
    # allocates registers.
    nc.compile()
    return nc


def _shard_inputs(v, v_pred):
    v = np.asarray(v, dtype=np.float32)
    v_pred = np.asarray(v_pred, dtype=np.float32)
    in_maps = []
    for c in range(NCORES):
        b, h = divmod(c, 2)
        xc = v_pred[b, h * XS:(h + 1) * XS]   # [4096, 3]
        y = v[b]                              # [8192, 3]
        in_maps.append({
            "xl": np.ascontiguousarray(xc.reshape(128, XGT * 3)),
            "yl": np.ascontiguousarray(y.reshape(128, YGT * 3)),
        })
    return in_maps


def _get_program():
    global _built
    if _built is None:
        _built = _build_program()
    return _built


def run_spmd(v, v_pred, **kwargs):
    """Run the SPMD program; returns BassKernelResults."""
    nc = _get_program()
    in_maps = _shard_inputs(v, v_pred)
    res = run_bass_kernel_spmd(nc, in_maps, list(range(NCORES)), **kwargs)
    return res


def kernel(v, v_pred):
    res = run_spmd(v, v_pred)
    total = 0.0
    for c in range(NCORES):
        total += np.asarray(res.results[c]["out"], dtype=np.float64).sum()
    mean = total / (B * NPTS)
    return np.array(mean, dtype=np.float32)
